# Initial kernel scaffold
#
"""CLIP 3D adapter kernel for 8x Trainium2 NeuronCores.

Strategy (view-sharded, sparse voxel table):
  - Core v owns view v (729 points, full 288 down-channels).
  - The G^3 voxel grid is ~95% empty (5832 points in 110592 voxels), so the
    dense value grid is never materialized. Instead:
      * slotptr: dense f32 map over a PADDED 50^3 grid (125000 cells) holding
        the canonical representative point id of each occupied voxel
        (1e9 = empty). Built by per-core scatter of own point ids +
        AllReduce(min). Padded border cells are never occupied, which makes
        conv3d zero-padding automatic.
      * T_sum [5888, 384] bf16: compact point-slot table of voxel sums
        (+count at col 288). Each core dma_scatter_adds its own 729 rows
        (duplicate indices accumulate exactly), then ONE AllReduce(add)
        merges cross-view partials ("segment_sum partials then psum").
      * T_vox [5888, 384] bf16 = sums / max(cnt, 1) (row 5832 stays zero and
        doubles as the "empty voxel" row).
  - conv3d+gather is reformulated point-wise: slot ids for all 27 neighbor
    offsets are fetched from slotptr with one indirect gather (OOB borders
    skip), then 27 transpose-mode dma_gathers pull voxel rows with CHANNELS
    ON PARTITIONS; DVE scales by w3 (per-partition scalars) and PE
    identity-matmuls accumulate in PSUM. gelu(tanh) on ACT out of PSUM.
  - conv2d depthwise 3x3 and the up-projection run directly on the
    channel-major layout (no transposes); residual is added in f32.

Wait legalization: this walrus path supports only ONE sync wait per
instruction, so split_multiwaits() hoists extra waits into standalone
event-semaphore ops after Tile scheduling.
"""

import numpy as np

import concourse.bass as bass
import concourse.mybir as mybir
from concourse.tile import TileContext
from concourse.bass_utils import run_bass_kernel_spmd
from concourse.masks import make_identity
from concourse import library_config
from concourse.library_overlay import lower_extended_insts

V, H, W, C = 8, 27, 27, 1152
L = H * W            # 729
Cd = C // 4          # 288
G = 48
GP = 50              # padded grid extent
NGRID = GP * GP * GP  # 125000
NGRID_AL = 125056    # 128 * 977, for the zero-fill DMA
P = 128
NP = 6               # point tiles per view (729 -> 768)
LPAD = NP * P        # 768
NPTS = V * L         # 5832
ZROW = NPTS          # the all-zero "empty voxel" table row
RROWS = 5888         # table rows: 46*128 (>= 5832); rows 5832+ stay zero
ELEM = 384           # table row width (288 data + cnt at 288 + pad) = 768B bf16
EMPTY_F = 1.0e9      # slotptr empty marker
CORE_IDS = list(range(8))

FP32 = mybir.dt.float32
BF16 = mybir.dt.bfloat16
I32 = mybir.dt.int32
I16 = mybir.dt.int16

_CACHED = {}


def split_multiwaits(nc):
    """This walrus path allows only ONE sync wait per instruction: hoist the
    extra waits into standalone InstEventSemaphore ops placed just before the
    owning instruction on the same engine stream."""
    for func in nc.m.functions:
        for block in func.blocks:
            out = []
            changed = False
            for inst in block.instructions:
                si = inst.sync_info
                if si is not None and si.on_wait and len(si.on_wait) > 1:
                    waits = list(si.on_wait)
                    for k, wt in enumerate(waits[:-1]):
                        out.append(mybir.InstEventSemaphore(
                            name=f"{inst.name}_w{k}",
                            engine=inst.engine,
                            ins=[], outs=[],
                            sync_info=mybir.SyncInfo(on_wait=[wt], on_update=[]),
                        ))
                    inst.sync_info = mybir.SyncInfo(
                        on_wait=[waits[-1]], on_update=list(si.on_update or []))
                    changed = True
                out.append(inst)
            if changed:
                block.instructions = out


def build_bass(split=True):
    nc = bass.Bass()

    # ---------------- I/O ----------------
    hs_nat = nc.dram_tensor("hs_nat", [LPAD, C], FP32, kind="ExternalInput")
    hsT = nc.dram_tensor("hsT", [C, LPAD], BF16, kind="ExternalInput")
    cw128 = nc.dram_tensor("cw128", [P, NP, 3], I32, kind="ExternalInput")
    cw16 = nc.dram_tensor("cw16", [16, 48, 3], I32, kind="ExternalInput")
    w_down = nc.dram_tensor("w_down", [C, Cd], BF16, kind="ExternalInput")
    w_up = nc.dram_tensor("w_up", [Cd, C], BF16, kind="ExternalInput")
    w3T = nc.dram_tensor("w3T", [P, 3, 27], FP32, kind="ExternalInput")
    w2col = nc.dram_tensor("w2col", [Cd, 9], FP32, kind="ExternalInput")
    pids = nc.dram_tensor("pids", [P, NP], FP32, kind="ExternalInput")
    out = nc.dram_tensor("out", [L, C], FP32, kind="ExternalOutput")

    # ---------------- internal DRAM ----------------
    slot_loc = nc.dram_tensor("slot_loc", [NGRID_AL], FP32)
    slot_sh = nc.dram_tensor("slot_sh", [NGRID_AL], FP32, addr_space="Shared")
    tsum_loc = nc.dram_tensor("tsum_loc", [RROWS, ELEM], BF16)
    tsum_sh = nc.dram_tensor("tsum_sh", [RROWS, ELEM], BF16, addr_space="Shared")
    tvox = nc.dram_tensor("tvox", [RROWS, ELEM], BF16)

    AO = mybir.AluOpType
    AF = mybir.ActivationFunctionType

    with TileContext(nc) as tc, \
         tc.tile_pool(name="persist", bufs=1) as pp, \
         tc.tile_pool(name="work", bufs=2) as wp, \
         tc.tile_pool(name="gwork", bufs=3) as gp, \
         tc.tile_pool(name="psum", bufs=2, space="PSUM") as psp, \
         tc.tile_pool(name="psacc", bufs=1, space="PSUM") as psa:

        # ================= A: loads + table init =================
        nc.gpsimd.load_library(library_config.mlp)
        hs_t = pp.tile([P, NP, C], FP32)
        nc.sync.dma_start(out=hs_t[:], in_=hs_nat[:].rearrange("(n p) c -> p n c", p=P))
        hsT_t = pp.tile([P, 9, LPAD], BF16)
        nc.sync.dma_start(out=hsT_t[:], in_=hsT[:].rearrange("(n p) l -> p n l", p=P))
        wd_t = pp.tile([P, 9, Cd], BF16)
        nc.sync.dma_start(out=wd_t[:], in_=w_down[:].rearrange("(n p) d -> p n d", p=P))
        wu_t = pp.tile([P, 3, C], BF16)
        nc.sync.dma_start(out=wu_t[:, :2, :], in_=w_up[:256, :].rearrange("(n p) c -> p n c", p=P))
        nc.sync.dma_start(out=wu_t[:32, 2, :], in_=w_up[256:, :])
        w3_t = pp.tile([P, 3, 27], FP32)
        nc.sync.dma_start(out=w3_t[:], in_=w3T[:])
        w2_t = pp.tile([P, 3, 9], FP32)
        nc.sync.dma_start(out=w2_t[:, :2, :], in_=w2col[:256, :].rearrange("(n p) k -> p n k", p=P))
        nc.sync.dma_start(out=w2_t[:32, 2, :], in_=w2col[256:, :])
        c128_t = pp.tile([P, NP, 3], I32)
        nc.sync.dma_start(out=c128_t[:], in_=cw128[:])
        c16_t = pp.tile([16, 48, 3], I32)
        nc.sync.dma_start(out=c16_t[:], in_=cw16[:])
        ids_f = pp.tile([P, NP], FP32)
        nc.sync.dma_start(out=ids_f[:], in_=pids[:])
        ident = pp.tile([P, P], BF16)
        make_identity(nc, ident[:])

        # table zero-fills (DVE memset -> 1-dep DMAs)
        zslot = pp.tile([P, NGRID_AL // P], FP32)   # 977 cols
        nc.vector.memset(zslot[:], EMPTY_F)
        dma_slot0 = nc.gpsimd.dma_start(
            out=slot_loc[:].rearrange("(p n) -> p n", p=P), in_=zslot[:])
        ztab = pp.tile([P, RROWS * ELEM // (2 * P)], BF16)  # 8832 cols
        nc.vector.memset(ztab[:], 0.0)
        HR = RROWS // 2
        ztab3 = ztab[:].rearrange("p (a e) -> p a e", e=ELEM)
        dma_tab0a = nc.gpsimd.dma_start(
            out=tsum_loc[:HR].rearrange("(a p) e -> p a e", p=P), in_=ztab3)
        dma_tab0b = nc.gpsimd.dma_start(
            out=tsum_loc[HR:].rearrange("(a p) e -> p a e", p=P), in_=ztab3)

        # ================= B1: down-projection =================
        src_t = pp.tile([P, NP, ELEM], BF16)
        nc.vector.memset(src_t[:], 0.0)
        nc.vector.memset(src_t[:, :, Cd:Cd + 1], 1.0)  # count channel
        for pt in range(NP):
            ps = psp.tile([P, Cd], FP32, tag="ps")
            for kt in range(9):
                nc.tensor.matmul(
                    out=ps[:],
                    lhsT=hsT_t[:, kt, pt * P:(pt + 1) * P],
                    rhs=wd_t[:, kt, :],
                    start=(kt == 0), stop=(kt == 8),
                )
            nc.scalar.activation(src_t[:, pt, 0:Cd], ps[:], AF.Copy)

        # ================= B2: flat ids + staging =================
        # padded-grid flat id: (x+1)*2500 + (y+1)*50 + (z+1)
        f128 = pp.tile([P, NP], I32)
        tmp0 = wp.tile([P, NP], I32, tag="ti")
        nc.vector.tensor_scalar_mul(out=f128[:], in0=c128_t[:, :, 0], scalar1=2500)
        nc.vector.tensor_scalar_mul(out=tmp0[:], in0=c128_t[:, :, 1], scalar1=50)
        nc.vector.tensor_tensor(out=f128[:], in0=f128[:], in1=tmp0[:], op=AO.add)
        nc.vector.tensor_tensor(out=f128[:], in0=f128[:], in1=c128_t[:, :, 2], op=AO.add)
        nc.vector.tensor_scalar_add(out=f128[:], in0=f128[:], scalar1=2551)


        # wrap-16 flat ids (48 cols = 768 slots; pads have coord 4096)
        f16 = pp.tile([16, 48], I32)
        tmp1 = wp.tile([16, 48], I32, tag="ti16")
        nc.vector.tensor_scalar_mul(out=f16[:], in0=c16_t[:, :, 0], scalar1=2500)
        nc.vector.tensor_scalar_mul(out=tmp1[:], in0=c16_t[:, :, 1], scalar1=50)
        nc.vector.tensor_tensor(out=f16[:], in0=f16[:], in1=tmp1[:], op=AO.add)
        nc.vector.tensor_tensor(out=f16[:], in0=f16[:], in1=c16_t[:, :, 2], op=AO.add)
        nc.vector.tensor_scalar_add(out=f16[:], in0=f16[:], scalar1=2551)
        valid16 = pp.tile([16, 48], I32)
        nc.vector.tensor_scalar(
            out=valid16[:], in0=f16[:], scalar1=NGRID, scalar2=None, op0=AO.is_lt)

        # 27 neighbor-offset target ids in wrap-16 layout [16, 27, 48]
        tflat = pp.tile([16, 27, 48], I32)
        offs = []
        for dx in (-1, 0, 1):
            for dy in (-1, 0, 1):
                for dz in (-1, 0, 1):
                    offs.append(dx * 2500 + dy * 50 + dz)
        for o, d in enumerate(offs):
            nc.vector.tensor_scalar_add(out=tflat[:, o, :], in0=f16[:], scalar1=d)

        # ================= B3: slotptr scatter + AllReduce(min) ============
        nc.gpsimd.indirect_dma_start(
            out=slot_loc[:, None],
            out_offset=bass.IndirectOffsetOnAxis(ap=f128[:, :], axis=0),
            in_=ids_f[:, :], in_offset=None,
            bounds_check=NGRID - 1, oob_is_err=False,
        )
        nc.gpsimd.collective_compute(
            "AllReduce", AO.min,
            replica_groups=[CORE_IDS],
            ins=[slot_loc[:]], outs=[slot_sh[:]],
        )

        # ================= B4: own-voxel slot gather -> int16 idx ==========
        s16f = pp.tile([16, 48], FP32)
        nc.vector.memset(s16f[:], 65536.0)
        nc.gpsimd.indirect_dma_start(
            out=s16f[:], out_offset=None,
            in_=slot_sh[:, None],
            in_offset=bass.IndirectOffsetOnAxis(ap=f16[:, :], axis=0),
            bounds_check=NGRID - 1, oob_is_err=False,
        )
        # s_sel = valid ? s : -1  (pads -> -1, trailing-ignored by scatter_add)
        s16sel = wp.tile([16, 48], FP32, tag="ts")
        vf = wp.tile([16, 48], FP32, tag="ts2")
        nc.vector.tensor_copy(out=vf[:], in_=valid16[:])
        nc.vector.tensor_tensor(out=s16sel[:], in0=s16f[:], in1=vf[:], op=AO.mult)
        nc.vector.tensor_scalar_add(out=vf[:], in0=vf[:], scalar1=-1.0)
        nc.vector.tensor_tensor(out=s16sel[:], in0=s16sel[:], in1=vf[:], op=AO.add)
        s16i = pp.tile([P, 48], I16)
        nc.vector.tensor_copy(out=s16i[:16, :], in_=s16sel[:])
        for k in range(1, 8):
            nc.gpsimd.dma_start(out=s16i[16 * k:16 * (k + 1), :], in_=s16i[:16, :])

        # ================= B5: local segment sums + AllReduce(add) =========
        nc.gpsimd.dma_scatter_add(
            tsum_loc[:],
            src_t[:],
            s16i[:, :46],
            736,
            L,      # num_idxs_reg = 729 (trailing pads are -1)
            ELEM,
        )
        nc.gpsimd.collective_compute(
            "AllReduce", AO.add,
            replica_groups=[CORE_IDS],
            ins=[tsum_loc[:]], outs=[tsum_sh[:]],
        )

        # ================= C1: divide -> T_vox =================
        dt = pp.tile([P, 46, ELEM], BF16)
        nc.sync.dma_start(
            out=dt[:], in_=tsum_sh[:].rearrange("(a p) e -> p a e", p=P))
        rc = wp.tile([P, 46], FP32, tag="rc")
        nc.vector.tensor_scalar_max(out=rc[:], in0=dt[:, :, Cd], scalar1=1.0)
        nc.vector.reciprocal(out=rc[:], in_=rc[:])
        for a in range(46):
            nc.vector.tensor_scalar_mul(
                out=dt[:, a, 0:Cd], in0=dt[:, a, 0:Cd], scalar1=rc[:, a:a + 1])
        nc.sync.dma_start(
            out=tvox[:].rearrange("(a p) e -> p a e", p=P), in_=dt[:])

        # ================= C2: neighbor slot ids for all 27 offsets ========
        so_all = pp.tile([16, 27, 48], FP32)
        nc.vector.memset(so_all[:], 65536.0)
        nc.gpsimd.indirect_dma_start(
            out=so_all[:].rearrange("p a b -> p (a b)"), out_offset=None,
            in_=slot_sh[:, None],
            in_offset=bass.IndirectOffsetOnAxis(
                ap=tflat[:].rearrange("p a b -> p (a b)"), axis=0),
            bounds_check=NGRID - 1, oob_is_err=False,
        )
        # clamp empties/pads to the zero row, convert to int16, replicate x8
        sall = pp.tile([16, 27 * 48], FP32)
        nc.vector.tensor_scalar_min(
            out=sall[:], in0=so_all[:].rearrange("p a b -> p (a b)"),
            scalar1=float(ZROW))
        s16g = pp.tile([P, 27 * 48], I16)
        nc.vector.tensor_copy(out=s16g[:16, :], in_=sall[:])
        for k in range(1, 8):
            nc.gpsimd.dma_start(out=s16g[16 * k:16 * (k + 1), :], in_=s16g[:16, :])

        # ================= C3: 27 transpose-gathers + combine ==============
        acc_ps = psa.tile([P, 3 * LPAD], FP32)   # 2304 f32 = 4.5 -> 5 banks
        for o in range(27):
            gt = gp.tile([P, 3, LPAD], BF16, tag="gt")
            nc.gpsimd.dma_gather(
                gt[:],
                tvox[:],
                s16g[:, o * 48:(o + 1) * 48],
                LPAD,
                LPAD,
                ELEM,
                transpose=True,
            )
            gs = gp.tile([P, 3, LPAD], BF16, tag="gs")
            for j in range(3):
                nc.vector.tensor_scalar_mul(
                    out=gs[:, j, :], in0=gt[:, j, :],
                    scalar1=w3_t[:, j, o:o + 1])
            for nb in range(5):
                lo = nb * 512
                hi = min((nb + 1) * 512, 3 * LPAD)
                if lo >= hi:
                    continue
                nc.tensor.matmul(
                    out=acc_ps[:, lo:hi],
                    lhsT=ident[:],
                    rhs=gs[:].rearrange("p a b -> p (a b)")[:, lo:hi],
                    start=(o == 0), stop=(o == 26),
                )

        # gelu tanh-approx composed from primitives (sim lacks Gelu_apprx_tanh):
        # gelu(x)*2 = x*(1+tanh(0.79788456*(x+0.044715*x^3))); the 0.5 is
        # folded into the conv2d weights on the host.
        xs = pp.tile([P, 3 * LPAD], BF16)
        nc.scalar.activation(xs[:], acc_ps[:], AF.Copy)
        x2 = wp.tile([P, 3 * LPAD], BF16, tag="gx")
        nc.vector.tensor_tensor(out=x2[:], in0=xs[:], in1=xs[:], op=AO.mult)
        nc.vector.tensor_scalar(
            out=x2[:], in0=x2[:], scalar1=0.044715, scalar2=1.0,
            op0=AO.mult, op1=AO.add)
        nc.vector.tensor_tensor(out=x2[:], in0=x2[:], in1=xs[:], op=AO.mult)
        th = wp.tile([P, 3 * LPAD], BF16, tag="gt2")
        nc.scalar.activation(th[:], x2[:], AF.Tanh, scale=0.7978845608028654)
        gact = pp.tile([P, 3, LPAD], BF16)
        nc.vector.scalar_tensor_tensor(
            out=gact[:].rearrange("p a b -> p (a b)"), in0=th[:], scalar=1.0,
            in1=xs[:], op0=AO.add, op1=AO.mult)

        # ================= C4: depthwise conv2d 3x3 on 27x27 ===============
        cv = pp.tile([P, 3, L], BF16)
        k = 0
        for dy in (-1, 0, 1):
            for dx in (-1, 0, 1):
                y0, y1 = max(0, -dy), min(H, H - dy)
                x0, x1 = max(0, -dx), min(W, W - dx)
                for ct in range(3):
                    cn = P if ct < 2 else 32
                    if k == 0:
                        nc.vector.memset(cv[:, ct, :], 0.0)
                    cv_ap = cv[:cn, ct, :].rearrange("p (y x) -> p y x", y=H)[
                        :, y0:y1, x0:x1]
                    in_ap = gact[:cn, ct, :L].rearrange("p (y x) -> p y x", y=H)[
                        :, y0 + dy:y1 + dy, x0 + dx:x1 + dx]
                    tmpm = wp.tile([P, H, W], BF16, tag="cvt")
                    nc.vector.tensor_scalar_mul(
                        out=tmpm[:cn, y0:y1, x0:x1], in0=in_ap,
                        scalar1=w2_t[:cn, ct, k:k + 1])
                    nc.vector.tensor_tensor(
                        out=cv_ap, in0=cv_ap, in1=tmpm[:cn, y0:y1, x0:x1], op=AO.add)
                k += 1

        # ================= C5: up-projection + residual ====================
        for pt in range(NP):
            rows = P if pt < 5 else L - 5 * P
            ot = wp.tile([P, C], FP32, tag="ot")
            for nchunk in range(3):
                ns = slice(nchunk * 384, (nchunk + 1) * 384)
                ps2 = psp.tile([P, 384], FP32, tag="ps")
                for ct in range(3):
                    cn = P if ct < 2 else 32
                    nc.tensor.matmul(
                        out=ps2[:rows, :], lhsT=cv[:cn, ct, pt * P:pt * P + rows],
                        rhs=wu_t[:cn, ct, ns],
                        start=(ct == 0), stop=(ct == 2),
                    )
                nc.vector.tensor_tensor(
                    out=ot[:rows, ns], in0=ps2[:rows, :],
                    in1=hs_t[:rows, pt, ns], op=AO.add)
            nc.sync.dma_start(out=out[pt * P:pt * P + rows, :], in_=ot[:rows, :])

    lower_extended_insts(nc)
    if split:
        split_multiwaits(nc)
    return nc


def _prep_inputs(hidden_states, coords, w_down, conv3d_w, conv2d_w, w_up):
    import ml_dtypes
    bf16 = ml_dtypes.bfloat16
    hs = np.asarray(hidden_states, np.float32)
    cd = np.asarray(coords, np.int32)
    w3 = np.asarray(conv3d_w, np.float32).reshape(Cd, 27)
    # w3T[part, j, o] = w3[j*128 + part, o], zero for invalid channels
    w3T = np.zeros((P, 3, 27), np.float32)
    for j in range(3):
        cn = min(P, Cd - j * P)
        w3T[:cn, j, :] = w3[j * P:j * P + cn, :]
    w2 = np.asarray(conv2d_w, np.float32).reshape(Cd, 9) * 0.5
    wd = np.asarray(w_down, np.float32).astype(bf16)
    wu = np.asarray(w_up, np.float32).astype(bf16)
    in_maps = []
    for v in range(V):
        hpad = np.zeros((LPAD, C), np.float32)
        hpad[:L] = hs[v]
        hsT = np.ascontiguousarray(hpad.T).astype(bf16)
        cv = cd[v].reshape(L, 3)
        cpad = np.full((LPAD, 3), 4096, np.int32)
        cpad[:L] = cv
        cw128 = np.ascontiguousarray(cpad.reshape(NP, P, 3).transpose(1, 0, 2))
        cw16 = np.ascontiguousarray(cpad.reshape(48, 16, 3).transpose(1, 0, 2))
        in_maps.append({
            "hs_nat": hpad,
            "hsT": hsT,
            "cw128": cw128,
            "cw16": cw16,
            "w_down": wd,
            "w_up": wu,
            "w3T": w3T,
            "w2col": w2,
            "pids": (v * L + np.arange(LPAD, dtype=np.float32)
                     ).reshape(NP, P).T.copy(),
        })
    return in_maps


def kernel(hidden_states, coords, w_down, conv3d_w, conv2d_w, w_up):
    if "nc" not in _CACHED:
        _CACHED["nc"] = build_bass()
    nc = _CACHED["nc"]
    in_maps = _prep_inputs(hidden_states, coords, w_down, conv3d_w, conv2d_w, w_up)
    res = run_bass_kernel_spmd(nc, in_maps, CORE_IDS)
    outs = [np.asarray(res.results[v]["out"], np.float32) for v in range(V)]
    return np.stack(outs, axis=0)



# revision 4
# speedup vs baseline: 1.7898x; 1.7898x over previous
"""CLIP 3D adapter kernel for 8x Trainium2 NeuronCores.

Strategy (view-sharded, sparse voxel table):
  - Core v owns view v (729 points, full 288 down-channels).
  - The G^3 voxel grid is ~95% empty (5832 points in 110592 voxels), so the
    dense value grid is never materialized. Instead:
      * slotptr: dense f32 map over a PADDED 50^3 grid (125000 cells) holding
        the canonical representative point id of each occupied voxel
        (1e9 = empty). Built by per-core scatter of own point ids +
        AllReduce(min). Padded border cells are never occupied, which makes
        conv3d zero-padding automatic.
      * T_sum [5888, 384] bf16: compact point-slot table of voxel sums
        (+count at col 288). Each core dma_scatter_adds its own 729 rows
        (duplicate indices accumulate exactly), then ONE AllReduce(add)
        merges cross-view partials ("segment_sum partials then psum").
      * T_vox [5888, 384] bf16 = sums / max(cnt, 1) (row 5832 stays zero and
        doubles as the "empty voxel" row).
  - conv3d+gather is reformulated point-wise: slot ids for all 27 neighbor
    offsets are fetched from slotptr with one indirect gather (OOB borders
    skip), then 27 transpose-mode dma_gathers pull voxel rows with CHANNELS
    ON PARTITIONS; DVE scales by w3 (per-partition scalars) and PE
    identity-matmuls accumulate in PSUM. gelu(tanh) on ACT out of PSUM.
  - conv2d depthwise 3x3 and the up-projection run directly on the
    channel-major layout (no transposes); residual is added in f32.

Wait legalization: this walrus path supports only ONE sync wait per
instruction, so split_multiwaits() hoists extra waits into standalone
event-semaphore ops after Tile scheduling.
"""

import numpy as np

import concourse.bass as bass
import concourse.mybir as mybir
from concourse.tile import TileContext
from concourse.bass_utils import run_bass_kernel_spmd
from concourse.masks import make_identity
from concourse import library_config
from concourse.library_overlay import lower_extended_insts

V, H, W, C = 8, 27, 27, 1152
L = H * W            # 729
Cd = C // 4          # 288
G = 48
GP = 50              # padded grid extent
NGRID = GP * GP * GP  # 125000
NGRID_AL = 125056    # 128 * 977, for the zero-fill DMA
P = 128
NP = 6               # point tiles per view (729 -> 768)
LPAD = NP * P        # 768
NPTS = V * L         # 5832
ZROW = NPTS          # the all-zero "empty voxel" table row
RROWS = 5888         # table rows: 46*128 (>= 5832); rows 5832+ stay zero
ELEM = 384           # table row width (288 data + cnt at 288 + pad) = 768B bf16
EMPTY_F = 1.0e9      # slotptr empty marker
CORE_IDS = list(range(8))

FP32 = mybir.dt.float32
BF16 = mybir.dt.bfloat16
I32 = mybir.dt.int32
I16 = mybir.dt.int16

_CACHED = {}


def split_multiwaits(nc):
    """This walrus path allows only ONE sync wait per instruction: hoist the
    extra waits into standalone InstEventSemaphore ops placed just before the
    owning instruction on the same engine stream."""
    for func in nc.m.functions:
        for block in func.blocks:
            out = []
            changed = False
            for inst in block.instructions:
                si = inst.sync_info
                if si is not None and si.on_wait and len(si.on_wait) > 1:
                    waits = list(si.on_wait)
                    for k, wt in enumerate(waits[:-1]):
                        out.append(mybir.InstEventSemaphore(
                            name=f"{inst.name}_w{k}",
                            engine=inst.engine,
                            ins=[], outs=[],
                            sync_info=mybir.SyncInfo(on_wait=[wt], on_update=[]),
                        ))
                    inst.sync_info = mybir.SyncInfo(
                        on_wait=[waits[-1]], on_update=list(si.on_update or []))
                    changed = True
                out.append(inst)
            if changed:
                block.instructions = out


def build_bass(split=True, collectives=True):
    nc = bass.Bass()

    # ---------------- I/O ----------------
    hs_nat = nc.dram_tensor("hs_nat", [LPAD, C], FP32, kind="ExternalInput")
    hsT = nc.dram_tensor("hsT", [C, LPAD], BF16, kind="ExternalInput")
    cw128 = nc.dram_tensor("cw128", [P, NP, 3], I32, kind="ExternalInput")
    cw16 = nc.dram_tensor("cw16", [16, 48, 3], I32, kind="ExternalInput")
    w_down = nc.dram_tensor("w_down", [C, Cd], BF16, kind="ExternalInput")
    w_up = nc.dram_tensor("w_up", [Cd, C], BF16, kind="ExternalInput")
    w3T = nc.dram_tensor("w3T", [P, 3, 27], FP32, kind="ExternalInput")
    w2col = nc.dram_tensor("w2col", [Cd, 9], FP32, kind="ExternalInput")
    pids = nc.dram_tensor("pids", [P, NP], FP32, kind="ExternalInput")
    out = nc.dram_tensor("out", [L, C], FP32, kind="ExternalOutput")

    # ---------------- internal DRAM ----------------
    slot_loc = nc.dram_tensor("slot_loc", [NGRID_AL], FP32)
    slot_sh = nc.dram_tensor("slot_sh", [NGRID_AL], FP32, addr_space="Shared")
    tsum_loc = nc.dram_tensor("tsum_loc", [RROWS, ELEM], BF16)
    tsum_sh = nc.dram_tensor("tsum_sh", [RROWS, ELEM], BF16, addr_space="Shared")
    tvox = nc.dram_tensor("tvox", [RROWS, ELEM], BF16)

    AO = mybir.AluOpType
    AF = mybir.ActivationFunctionType

    with TileContext(nc) as tc, \
         tc.tile_pool(name="persist", bufs=1) as pp, \
         tc.tile_pool(name="work", bufs=2) as wp, \
         tc.tile_pool(name="gwork", bufs=3) as gp, \
         tc.tile_pool(name="psum", bufs=2, space="PSUM") as psp, \
         tc.tile_pool(name="psacc", bufs=1, space="PSUM") as psa:

        # ================= A: loads + table init =================
        nc.gpsimd.load_library(library_config.mlp)
        hs_t = pp.tile([P, NP, C], FP32)
        nc.sync.dma_start(out=hs_t[:], in_=hs_nat[:].rearrange("(n p) c -> p n c", p=P))
        hsT_t = pp.tile([P, 9, LPAD], BF16)
        nc.sync.dma_start(out=hsT_t[:], in_=hsT[:].rearrange("(n p) l -> p n l", p=P))
        wd_t = pp.tile([P, 9, Cd], BF16)
        nc.sync.dma_start(out=wd_t[:], in_=w_down[:].rearrange("(n p) d -> p n d", p=P))
        wu_t = pp.tile([P, 3, C], BF16)
        nc.sync.dma_start(out=wu_t[:, :2, :], in_=w_up[:256, :].rearrange("(n p) c -> p n c", p=P))
        nc.sync.dma_start(out=wu_t[:32, 2, :], in_=w_up[256:, :])
        w3_t = pp.tile([P, 3, 27], FP32)
        nc.sync.dma_start(out=w3_t[:], in_=w3T[:])
        w2_t = pp.tile([P, 3, 9], FP32)
        nc.sync.dma_start(out=w2_t[:, :2, :], in_=w2col[:256, :].rearrange("(n p) k -> p n k", p=P))
        nc.sync.dma_start(out=w2_t[:32, 2, :], in_=w2col[256:, :])
        c128_t = pp.tile([P, NP, 3], I32)
        nc.sync.dma_start(out=c128_t[:], in_=cw128[:])
        c16_t = pp.tile([16, 48, 3], I32)
        nc.sync.dma_start(out=c16_t[:], in_=cw16[:])
        ids_f = pp.tile([P, NP], FP32)
        nc.sync.dma_start(out=ids_f[:], in_=pids[:])
        ident = pp.tile([P, P], BF16)
        make_identity(nc, ident[:])

        # table zero-fills (DVE memset -> 1-dep DMAs)
        zslot = pp.tile([P, NGRID_AL // P], FP32)   # 977 cols
        nc.vector.memset(zslot[:], EMPTY_F)
        dma_slot0 = nc.gpsimd.dma_start(
            out=slot_loc[:].rearrange("(p n) -> p n", p=P), in_=zslot[:])
        ztab = pp.tile([P, RROWS * ELEM // (2 * P)], BF16)  # 8832 cols
        nc.vector.memset(ztab[:], 0.0)
        HR = RROWS // 2
        ztab3 = ztab[:].rearrange("p (a e) -> p a e", e=ELEM)
        dma_tab0a = nc.gpsimd.dma_start(
            out=tsum_loc[:HR].rearrange("(a p) e -> p a e", p=P), in_=ztab3)
        dma_tab0b = nc.gpsimd.dma_start(
            out=tsum_loc[HR:].rearrange("(a p) e -> p a e", p=P), in_=ztab3)

        # ================= B1: down-projection =================
        src_t = pp.tile([P, NP, ELEM], BF16)
        nc.vector.memset(src_t[:], 0.0)
        nc.vector.memset(src_t[:, :, Cd:Cd + 1], 1.0)  # count channel
        for pt in range(NP):
            ps = psp.tile([P, Cd], FP32, tag="ps")
            for kt in range(9):
                nc.tensor.matmul(
                    out=ps[:],
                    lhsT=hsT_t[:, kt, pt * P:(pt + 1) * P],
                    rhs=wd_t[:, kt, :],
                    start=(kt == 0), stop=(kt == 8),
                )
            nc.scalar.activation(src_t[:, pt, 0:Cd], ps[:], AF.Copy)

        # ================= B2: flat ids + staging =================
        # padded-grid flat id: (x+1)*2500 + (y+1)*50 + (z+1)
        f128 = pp.tile([P, NP], I32)
        tmp0 = wp.tile([P, NP], I32, tag="ti")
        nc.vector.tensor_scalar_mul(out=f128[:], in0=c128_t[:, :, 0], scalar1=2500)
        nc.vector.tensor_scalar_mul(out=tmp0[:], in0=c128_t[:, :, 1], scalar1=50)
        nc.vector.tensor_tensor(out=f128[:], in0=f128[:], in1=tmp0[:], op=AO.add)
        nc.vector.tensor_tensor(out=f128[:], in0=f128[:], in1=c128_t[:, :, 2], op=AO.add)
        nc.vector.tensor_scalar_add(out=f128[:], in0=f128[:], scalar1=2551)


        # wrap-16 flat ids (48 cols = 768 slots; pads have coord 4096)
        f16 = pp.tile([16, 48], I32)
        tmp1 = wp.tile([16, 48], I32, tag="ti16")
        nc.vector.tensor_scalar_mul(out=f16[:], in0=c16_t[:, :, 0], scalar1=2500)
        nc.vector.tensor_scalar_mul(out=tmp1[:], in0=c16_t[:, :, 1], scalar1=50)
        nc.vector.tensor_tensor(out=f16[:], in0=f16[:], in1=tmp1[:], op=AO.add)
        nc.vector.tensor_tensor(out=f16[:], in0=f16[:], in1=c16_t[:, :, 2], op=AO.add)
        nc.vector.tensor_scalar_add(out=f16[:], in0=f16[:], scalar1=2551)
        valid16 = pp.tile([16, 48], I32)
        nc.vector.tensor_scalar(
            out=valid16[:], in0=f16[:], scalar1=NGRID, scalar2=None, op0=AO.is_lt)

        # 27 neighbor-offset target ids in wrap-16 layout [16, 27, 48]
        tflat = pp.tile([16, 27, 48], I32)
        offs = []
        for dx in (-1, 0, 1):
            for dy in (-1, 0, 1):
                for dz in (-1, 0, 1):
                    offs.append(dx * 2500 + dy * 50 + dz)
        for o, d in enumerate(offs):
            nc.vector.tensor_scalar_add(out=tflat[:, o, :], in0=f16[:], scalar1=d)

        # ================= B3: slotptr scatter + AllReduce(min) ============
        nc.gpsimd.indirect_dma_start(
            out=slot_loc[:, None],
            out_offset=bass.IndirectOffsetOnAxis(ap=f128[:, :], axis=0),
            in_=ids_f[:, :], in_offset=None,
            bounds_check=NGRID - 1, oob_is_err=False,
        )
        if collectives:
            nc.gpsimd.collective_compute(
                "AllReduce", AO.min,
                replica_groups=[CORE_IDS],
                ins=[slot_loc[:]], outs=[slot_sh[:]],
            )
        else:
            nc.gpsimd.dma_start(out=slot_sh[:], in_=slot_loc[:])

        # ================= B4: own-voxel slot gather -> int16 idx ==========
        s16f = pp.tile([16, 48], FP32)
        nc.vector.memset(s16f[:], 65536.0)
        nc.gpsimd.indirect_dma_start(
            out=s16f[:], out_offset=None,
            in_=slot_sh[:, None],
            in_offset=bass.IndirectOffsetOnAxis(ap=f16[:, :], axis=0),
            bounds_check=NGRID - 1, oob_is_err=False,
        )
        # s_sel = valid ? s : -1  (pads -> -1, trailing-ignored by scatter_add)
        s16sel = wp.tile([16, 48], FP32, tag="ts")
        vf = wp.tile([16, 48], FP32, tag="ts2")
        nc.vector.tensor_copy(out=vf[:], in_=valid16[:])
        nc.vector.tensor_tensor(out=s16sel[:], in0=s16f[:], in1=vf[:], op=AO.mult)
        nc.vector.tensor_scalar_add(out=vf[:], in0=vf[:], scalar1=-1.0)
        nc.vector.tensor_tensor(out=s16sel[:], in0=s16sel[:], in1=vf[:], op=AO.add)
        s16i = pp.tile([P, 48], I16)
        nc.vector.tensor_copy(out=s16i[:16, :], in_=s16sel[:])
        for k in range(1, 8):
            nc.gpsimd.dma_start(out=s16i[16 * k:16 * (k + 1), :], in_=s16i[:16, :])

        # ================= B5: local segment sums + AllReduce(add) =========
        nc.gpsimd.dma_scatter_add(
            tsum_loc[:],
            src_t[:],
            s16i[:, :46],
            736,
            L,      # num_idxs_reg = 729 (trailing pads are -1)
            ELEM,
        )
        if collectives:
            nc.gpsimd.collective_compute(
                "AllReduce", AO.add,
                replica_groups=[CORE_IDS],
                ins=[tsum_loc[:]], outs=[tsum_sh[:]],
            )
        else:
            nc.gpsimd.dma_start(out=tsum_sh[:], in_=tsum_loc[:])

        # ================= C1: divide -> T_vox =================
        dt = pp.tile([P, 46, ELEM], BF16)
        nc.sync.dma_start(
            out=dt[:], in_=tsum_sh[:].rearrange("(a p) e -> p a e", p=P))
        rc = wp.tile([P, 46], FP32, tag="rc")
        nc.vector.tensor_scalar_max(out=rc[:], in0=dt[:, :, Cd], scalar1=1.0)
        nc.vector.reciprocal(out=rc[:], in_=rc[:])
        for a in range(46):
            nc.vector.tensor_scalar_mul(
                out=dt[:, a, 0:Cd], in0=dt[:, a, 0:Cd], scalar1=rc[:, a:a + 1])
        nc.sync.dma_start(
            out=tvox[:].rearrange("(a p) e -> p a e", p=P), in_=dt[:])

        # ================= C2: neighbor slot ids for all 27 offsets ========
        so_all = pp.tile([16, 27, 48], FP32)
        nc.vector.memset(so_all[:], 65536.0)
        nc.gpsimd.indirect_dma_start(
            out=so_all[:].rearrange("p a b -> p (a b)"), out_offset=None,
            in_=slot_sh[:, None],
            in_offset=bass.IndirectOffsetOnAxis(
                ap=tflat[:].rearrange("p a b -> p (a b)"), axis=0),
            bounds_check=NGRID - 1, oob_is_err=False,
        )
        # clamp empties/pads to the zero row, convert to int16, replicate x8
        sall = pp.tile([16, 27 * 48], FP32)
        nc.vector.tensor_scalar_min(
            out=sall[:], in0=so_all[:].rearrange("p a b -> p (a b)"),
            scalar1=float(ZROW))
        s16g = pp.tile([P, 27 * 48], I16)
        nc.vector.tensor_copy(out=s16g[:16, :], in_=sall[:])
        for k in range(1, 8):
            nc.gpsimd.dma_start(out=s16g[16 * k:16 * (k + 1), :], in_=s16g[:16, :])

        # ================= C3: 27 transpose-gathers + combine ==============
        acc_ps = psa.tile([P, 3 * LPAD], FP32)   # 2304 f32 = 4.5 -> 5 banks
        for o in range(27):
            gt = gp.tile([P, 3, LPAD], BF16, tag="gt")
            nc.gpsimd.dma_gather(
                gt[:],
                tvox[:],
                s16g[:, o * 48:(o + 1) * 48],
                LPAD,
                LPAD,
                ELEM,
                transpose=True,
            )
            gs = gp.tile([P, 3, LPAD], BF16, tag="gs")
            for j in range(3):
                nc.vector.tensor_scalar_mul(
                    out=gs[:, j, :], in0=gt[:, j, :],
                    scalar1=w3_t[:, j, o:o + 1])
            for nb in range(5):
                lo = nb * 512
                hi = min((nb + 1) * 512, 3 * LPAD)
                if lo >= hi:
                    continue
                nc.tensor.matmul(
                    out=acc_ps[:, lo:hi],
                    lhsT=ident[:],
                    rhs=gs[:].rearrange("p a b -> p (a b)")[:, lo:hi],
                    start=(o == 0), stop=(o == 26),
                )

        # gelu tanh-approx composed from primitives (sim lacks Gelu_apprx_tanh):
        # gelu(x)*2 = x*(1+tanh(0.79788456*(x+0.044715*x^3))); the 0.5 is
        # folded into the conv2d weights on the host.
        xs = pp.tile([P, 3 * LPAD], BF16)
        nc.scalar.activation(xs[:], acc_ps[:], AF.Copy)
        x2 = wp.tile([P, 3 * LPAD], BF16, tag="gx")
        nc.vector.tensor_tensor(out=x2[:], in0=xs[:], in1=xs[:], op=AO.mult)
        nc.vector.tensor_scalar(
            out=x2[:], in0=x2[:], scalar1=0.044715, scalar2=1.0,
            op0=AO.mult, op1=AO.add)
        nc.vector.tensor_tensor(out=x2[:], in0=x2[:], in1=xs[:], op=AO.mult)
        th = wp.tile([P, 3 * LPAD], BF16, tag="gt2")
        nc.scalar.activation(th[:], x2[:], AF.Tanh, scale=0.7978845608028654)
        gact = pp.tile([P, 3, LPAD], BF16)
        nc.vector.scalar_tensor_tensor(
            out=gact[:].rearrange("p a b -> p (a b)"), in0=th[:], scalar=1.0,
            in1=xs[:], op0=AO.add, op1=AO.mult)

        # ================= C4: depthwise conv2d 3x3 on 27x27 ===============
        cv = pp.tile([P, 3, L], BF16)
        k = 0
        for dy in (-1, 0, 1):
            for dx in (-1, 0, 1):
                y0, y1 = max(0, -dy), min(H, H - dy)
                x0, x1 = max(0, -dx), min(W, W - dx)
                for ct in range(3):
                    cn = P if ct < 2 else 32
                    if k == 0:
                        nc.vector.memset(cv[:, ct, :], 0.0)
                    cv_ap = cv[:cn, ct, :].rearrange("p (y x) -> p y x", y=H)[
                        :, y0:y1, x0:x1]
                    in_ap = gact[:cn, ct, :L].rearrange("p (y x) -> p y x", y=H)[
                        :, y0 + dy:y1 + dy, x0 + dx:x1 + dx]
                    tmpm = wp.tile([P, H, W], BF16, tag="cvt")
                    nc.vector.tensor_scalar_mul(
                        out=tmpm[:cn, y0:y1, x0:x1], in0=in_ap,
                        scalar1=w2_t[:cn, ct, k:k + 1])
                    nc.vector.tensor_tensor(
                        out=cv_ap, in0=cv_ap, in1=tmpm[:cn, y0:y1, x0:x1], op=AO.add)
                k += 1

        # ================= C5: up-projection + residual ====================
        for pt in range(NP):
            rows = P if pt < 5 else L - 5 * P
            ot = wp.tile([P, C], FP32, tag="ot")
            for nchunk in range(3):
                ns = slice(nchunk * 384, (nchunk + 1) * 384)
                ps2 = psp.tile([P, 384], FP32, tag="ps")
                for ct in range(3):
                    cn = P if ct < 2 else 32
                    nc.tensor.matmul(
                        out=ps2[:rows, :], lhsT=cv[:cn, ct, pt * P:pt * P + rows],
                        rhs=wu_t[:cn, ct, ns],
                        start=(ct == 0), stop=(ct == 2),
                    )
                nc.vector.tensor_tensor(
                    out=ot[:rows, ns], in0=ps2[:rows, :],
                    in1=hs_t[:rows, pt, ns], op=AO.add)
            nc.sync.dma_start(out=out[pt * P:pt * P + rows, :], in_=ot[:rows, :])

    lower_extended_insts(nc)
    if split:
        split_multiwaits(nc)
    return nc


def _prep_inputs(hidden_states, coords, w_down, conv3d_w, conv2d_w, w_up):
    import ml_dtypes
    bf16 = ml_dtypes.bfloat16
    hs = np.asarray(hidden_states, np.float32)
    cd = np.asarray(coords, np.int32)
    w3 = np.asarray(conv3d_w, np.float32).reshape(Cd, 27)
    # w3T[part, j, o] = w3[j*128 + part, o], zero for invalid channels
    w3T = np.zeros((P, 3, 27), np.float32)
    for j in range(3):
        cn = min(P, Cd - j * P)
        w3T[:cn, j, :] = w3[j * P:j * P + cn, :]
    w2 = np.asarray(conv2d_w, np.float32).reshape(Cd, 9) * 0.5
    wd = np.asarray(w_down, np.float32).astype(bf16)
    wu = np.asarray(w_up, np.float32).astype(bf16)
    in_maps = []
    for v in range(V):
        hpad = np.zeros((LPAD, C), np.float32)
        hpad[:L] = hs[v]
        hsT = np.ascontiguousarray(hpad.T).astype(bf16)
        cv = cd[v].reshape(L, 3)
        cpad = np.full((LPAD, 3), 4096, np.int32)
        cpad[:L] = cv
        cw128 = np.ascontiguousarray(cpad.reshape(NP, P, 3).transpose(1, 0, 2))
        cw16 = np.ascontiguousarray(cpad.reshape(48, 16, 3).transpose(1, 0, 2))
        in_maps.append({
            "hs_nat": hpad,
            "hsT": hsT,
            "cw128": cw128,
            "cw16": cw16,
            "w_down": wd,
            "w_up": wu,
            "w3T": w3T,
            "w2col": w2,
            "pids": (v * L + np.arange(LPAD, dtype=np.float32)
                     ).reshape(NP, P).T.copy(),
        })
    return in_maps


def kernel(hidden_states, coords, w_down, conv3d_w, conv2d_w, w_up):
    if "nc" not in _CACHED:
        _CACHED["nc"] = build_bass()
    nc = _CACHED["nc"]
    in_maps = _prep_inputs(hidden_states, coords, w_down, conv3d_w, conv2d_w, w_up)
    res = run_bass_kernel_spmd(nc, in_maps, CORE_IDS)
    outs = [np.asarray(res.results[v]["out"], np.float32) for v in range(V)]
    return np.stack(outs, axis=0)

